# revision 1
# baseline (speedup 1.0000x reference)
"""Trainium2 Bass kernel for nn_Attention_40037685133427.

FiLM-conditioned LayerNorm + 16-head self-attention (B=2, N=2048, D=1024),
tensor-parallel over 8 NeuronCores: core c owns heads {2c, 2c+1}.

Per-core dataflow (transposed-native [feature, token] layouts, bf16 compute
with fp32 PSUM accumulation; host pre-casts x^T and weights to bf16):
  - LN stats via PE ones-matmuls (cross-partition sums over the model dim),
    rstd by DVE-only Newton rsqrt (keeps ACT on one exp table set);
    per-token u=rstd and m=mean*rstd broadcast across partitions by Kc=1
    matmuls, staged so round-trips never block an engine queue head.
  - The LN+FiLM affine is folded into the QKV weights (gamma'-scaled,
    per batch) plus a 3-op per-token correction applied to the QKV
    *outputs* (3x less elementwise work than normalizing x), so the QKV
    matmuls run on raw x and never wait for the stats round-trip.
  - V re-transposed to natural layout via PE transpose; h0's V carries an
    extra ones column so attn@V also produces h0's softmax denominator.
  - attention: S^T = K Q^T with two heads row-tiled into one 2-bank PSUM
    tile, a single fused exp per (jt, islice) on ACT (scale=1/sqrt(dh)
    folded in; no max subtraction - |S| < ~4 by construction), col-tiled
    attn@V, plus a ones-matmul for h1's denominator.
  - softmax normalization fused into the PSUM->SBUF evacuation via a
    PE-broadcast reciprocal tile; both batches' attention issue before
    either normalize so the denominator round-trip hides under compute.
  - y^T = Wo^T-layout matmul over the fused 128-wide head slice.
Host sums the 8 partial y^T outputs (row-split Wo => partial sums).
Measured: 540.7 us HW exec per core, rel L2 error 0.005 vs fp32 reference.
"""

import sys

sys.path.insert(0, "/opt/trn_rl_repo")

import numpy as np
import ml_dtypes

import concourse.bass as bass
from concourse import bacc
import concourse.tile as tile
from concourse import mybir
from concourse.bass_utils import run_bass_kernel_spmd
from concourse.masks import make_identity

f32 = mybir.dt.float32
bf16 = mybir.dt.bfloat16
AF = mybir.ActivationFunctionType
ALU = mybir.AluOpType

B, N, DIM = 2, 2048, 1024
HEADS, DH = 16, 64
TOK = B * N            # 4096 tokens, batch-major
KT = DIM // 128        # 8 k-tiles over the model dim
NSL = TOK // 512       # 8 token slices of 512
JT = N // 128          # 16 key tiles per batch
COND = 1024
NCORES = 8


def build_program():
    nc = bacc.Bacc("TRN2", target_bir_lowering=False, debug=False)

    xT = nc.dram_tensor("xT", [DIM, TOK], bf16, kind="ExternalInput").ap()
    ceT = nc.dram_tensor("ceT", [128, 2 * KT], f32, kind="ExternalInput").ap()
    gammaT = nc.dram_tensor("gammaT", [128, KT], f32, kind="ExternalInput").ap()
    condW = nc.dram_tensor("condW", [COND, 2 * DIM], bf16, kind="ExternalInput").ap()
    condb = nc.dram_tensor("condb", [2, 2 * DIM], f32, kind="ExternalInput").ap()
    wqkv = nc.dram_tensor("wqkv", [DIM, 384], bf16, kind="ExternalInput").ap()
    wo = nc.dram_tensor("wo", [128, DIM], bf16, kind="ExternalInput").ap()
    ones2_in = nc.dram_tensor("ones2", [2, 128], bf16, kind="ExternalInput").ap()

    yT_out = nc.dram_tensor("yT", [DIM, TOK], bf16, kind="ExternalOutput").ap()

    # internal DRAM bounce buffers
    film_d = nc.dram_tensor("film_d", [2, 2, KT, 128], f32).ap()   # (b, scale/shift, kt, p)
    stats_d = nc.dram_tensor("stats_d", [2, TOK], f32).ap()        # (sum|sumsq, tok)
    um_d = nc.dram_tensor("um_d", [2, TOK], bf16).ap()             # (u|m, tok)
    den_d = nc.dram_tensor("den_d", [B, 4, 2, 512], f32).ap()      # (b, isl, h, x)
    r_d = nc.dram_tensor("r_d", [B, 4, 2, 512], bf16).ap()
    wsum_d = nc.dram_tensor("wsum_d", [B, 2, 384], f32).ap()

    with tile.TileContext(nc) as tc:
        with (
            tc.tile_pool(name="const", bufs=1) as const,
            tc.tile_pool(name="persist", bufs=1) as persist,
            tc.tile_pool(name="big", bufs=1) as bigp,
            tc.tile_pool(name="work", bufs=3) as work,
            tc.tile_pool(name="ps", bufs=8, space="PSUM") as ps,
        ):
            def pst(shape=(128, 512), dtype=f32):
                return ps.tile(list(shape), dtype, tag="ps", bufs=4, name="pstile")

            def pst2():
                return ps.tile([128, 1024], f32, tag="st2", bufs=2, name="st2tile")

            def b512(name):
                # shared 128KB-slot pool: x tiles first, P^T tiles reuse after QKV
                return bigp.tile([128, 512], bf16, tag="b512", bufs=64, name=name)

            # ---------------- constants / weights ----------------
            ident = const.tile([128, 128], bf16)
            make_identity(nc, ident[:])
            ones_col = const.tile([128, 1], bf16)
            nc.vector.memset(ones_col[:], 1.0)
            ones1 = const.tile([1, 128], bf16)
            nc.vector.memset(ones1[:], 1.0)
            ones2 = const.tile([2, 128], bf16)
            nc.gpsimd.dma_start(ones2[:], ones2_in)

            wo_bf = persist.tile([128, DIM], bf16, tag="wo")
            nc.sync.dma_start(wo_bf[:], wo)

            gam = const.tile([128, KT], f32)
            nc.gpsimd.dma_start(gam[:], gammaT)
            cet = const.tile([128, 2 * KT], f32)
            nc.gpsimd.dma_start(cet[:], ceT)

            # ---------------- FiLM conditioning (gates the film stage) ----------------
            sil = const.tile([128, 2 * KT], f32)
            # silu(x) = x / (1 + exp(-x)) -- via Exp so a single ACT table set is used
            nc.scalar.activation(sil[:], cet[:], AF.Exp, scale=-1.0)
            nc.vector.tensor_scalar(sil[:], sil[:], 1.0, None, ALU.add)
            nc.vector.reciprocal(sil[:], sil[:])
            nc.vector.tensor_tensor(sil[:], sil[:], cet[:], op=ALU.mult)
            sil_bf = const.tile([128, 2 * KT], bf16)
            nc.vector.tensor_copy(sil_bf[:], sil[:])
            film_flat = film_d.rearrange("b s k p -> b (s k p)")
            for cs in range(4):
                pc = pst((2, 512))
                for kt in range(KT):
                    cw = work.tile([128, 512], bf16, tag="cw", bufs=3)
                    nc.sync.dma_start(cw[:], condW[kt * 128:(kt + 1) * 128, cs * 512:(cs + 1) * 512])
                    nc.tensor.matmul(pc[:], sil_bf[:, 2 * kt:2 * kt + 2], cw[:],
                                     start=(kt == 0), stop=(kt == KT - 1))
                sl = slice(cs * 512, (cs + 1) * 512)
                cbw = work.tile([2, 512], f32, tag="cbw", bufs=1)
                nc.gpsimd.dma_start(cbw[:], condb[:, sl])
                csl = work.tile([2, 512], f32, tag="csl", bufs=1)
                nc.vector.tensor_tensor(csl[:], pc[:], cbw[:], op=ALU.add)
                nc.gpsimd.dma_start(film_flat[:, sl], csl[:])
            gp = const.tile([128, 2 * KT], f32)   # gamma' columns, col = b*KT + kt
            bp = const.tile([128, 2 * KT], f32)   # beta'
            for b in range(B):
                sl = slice(b * KT, (b + 1) * KT)
                nc.gpsimd.dma_start(gp[:, sl], film_d[b, 0].rearrange("k p -> p k"))
                nc.gpsimd.dma_start(bp[:, sl], film_d[b, 1].rearrange("k p -> p k"))
            gpf = const.tile([128, 2 * KT], f32)
            nc.vector.tensor_scalar(gpf[:], gp[:], 1.0, None, ALU.add)
            for b in range(B):
                sl = slice(b * KT, (b + 1) * KT)
                nc.vector.tensor_tensor(gpf[:, sl], gpf[:, sl], gam[:], op=ALU.mult)
            eps_t = const.tile([128, 1], f32)
            nc.vector.memset(eps_t[:], 1e-5)
            # ---------------- LN stats, software-pipelined in stages ----------------
            x_bf = [[None] * NSL for _ in range(KT)]   # [kt][isl] -> [128,512] bf16
            U_sb, MU_sb = [None] * NSL, [None] * NSL
            umT = um_d.rearrange("s (C p) -> s p C", p=128)
            # stage 1: loads + x^2 + cross-partition sums -> stats_d
            for isl in range(NSL):
                sl = slice(isl * 512, (isl + 1) * 512)
                psu = pst((1, 512))
                psq = pst((1, 512))
                for kt in range(KT):
                    xb = b512(f"x{kt}_{isl}")
                    nc.sync.dma_start(xb[:], xT[kt * 128:(kt + 1) * 128, sl])
                    x_bf[kt][isl] = xb
                    xsq = work.tile([128, 512], bf16, tag="xsq", bufs=2)
                    nc.vector.tensor_tensor(xsq[:], xb[:], xb[:], op=ALU.mult)
                    nc.tensor.matmul(psu[:], ones_col[:], xb[:],
                                     start=(kt == 0), stop=(kt == KT - 1))
                    nc.tensor.matmul(psq[:], ones_col[:], xsq[:],
                                     start=(kt == 0), stop=(kt == KT - 1))
                surow = work.tile([1, 512], f32, tag="statrow", bufs=2)
                nc.vector.tensor_copy(surow[:], psu[:])
                nc.sync.dma_start(stats_d[0:1, sl], surow[:])
                sqrow = work.tile([1, 512], f32, tag="statrow", bufs=2)
                nc.vector.tensor_copy(sqrow[:], psq[:])
                nc.sync.dma_start(stats_d[1:2, sl], sqrow[:])
            bpb = const.tile([128, 2 * KT], bf16)
            nc.vector.tensor_copy(bpb[:], bp[:])
            # per-batch gamma'-scaled QKV weights + per-output-column sums:
            #   q_film^T = U * (W_g^T x^T) - (M*U) * sum_d(W_g) + sum_d(beta' W)
            wscaled = []
            for b in range(B):
                wsb = []
                pgs = pst((1, 512))
                pbs = pst((1, 512))
                for kt in range(KT):
                    col = b * KT + kt
                    wg = persist.tile([128, 384], bf16, tag="wg", bufs=2 * KT)
                    nc.sync.dma_start(wg[:], wqkv[kt * 128:(kt + 1) * 128, :])
                    nc.tensor.matmul(pbs[0:1, 0:384], bpb[:, col:col + 1], wg[:],
                                     start=(kt == 0), stop=(kt == KT - 1))
                    nc.vector.tensor_scalar(wg[:], wg[:], gpf[:, col:col + 1], None, ALU.mult)
                    nc.tensor.matmul(pgs[0:1, 0:384], ones_col[:], wg[:],
                                     start=(kt == 0), stop=(kt == KT - 1))
                    wsb.append(wg)
                wscaled.append(wsb)
                gsr = work.tile([1, 512], f32, tag="statrow", bufs=2)
                nc.vector.tensor_copy(gsr[0:1, 0:384], pgs[0:1, 0:384])
                nc.gpsimd.dma_start(wsum_d[b, 0], gsr[0:1, 0:384])
                bsr = work.tile([1, 512], f32, tag="statrow", bufs=2)
                nc.vector.tensor_copy(bsr[0:1, 0:384], pbs[0:1, 0:384])
                nc.gpsimd.dma_start(wsum_d[b, 1], bsr[0:1, 0:384])
            wgs_neg, wbs = [], []
            for b in range(B):
                wg_n = const.tile([128, 3], f32, name=f"wgn{b}")
                nc.gpsimd.dma_start(wg_n[:], wsum_d[b, 0].rearrange("(c p) -> p c", p=128))
                nc.vector.tensor_scalar(wg_n[:], wg_n[:], -1.0, None, ALU.mult)
                wgs_neg.append(wg_n)
                wb_c = const.tile([128, 3], f32, name=f"wbc{b}")
                nc.gpsimd.dma_start(wb_c[:], wsum_d[b, 1].rearrange("(c p) -> p c", p=128))
                wbs.append(wb_c)

            # stage 2: per-token mean/var -> rstd -> um_d
            for isl in range(NSL):
                sl = slice(isl * 512, (isl + 1) * 512)
                sc = work.tile([128, 8], f32, tag="sc", bufs=4)
                nc.sync.dma_start(sc[:, 0:4], stats_d[0:1, sl].rearrange("s (c p) -> p s c", p=128))
                nc.sync.dma_start(sc[:, 4:8], stats_d[1:2, sl].rearrange("s (c p) -> p s c", p=128))
                mean_t = work.tile([128, 4], f32, tag="mean", bufs=4)
                var_t = work.tile([128, 4], f32, tag="var", bufs=4)
                nc.vector.tensor_scalar(mean_t[:], sc[:, 0:4], 1.0 / DIM, None, ALU.mult)
                nc.vector.tensor_scalar(var_t[:], sc[:, 4:8], 1.0 / DIM, None, ALU.mult)
                msq = work.tile([128, 4], f32, tag="msq", bufs=4)
                nc.vector.tensor_tensor(msq[:], mean_t[:], mean_t[:], op=ALU.mult)
                nc.vector.tensor_tensor(var_t[:], var_t[:], msq[:], op=ALU.subtract)
                nc.vector.tensor_scalar(var_t[:], var_t[:], 1e-5, None, ALU.add)
                # rstd = rsqrt(var) by Newton: y0 = 1.5 - 0.5 v; y <- y*(1.5 - 0.5*v*y^2)
                u_t = work.tile([128, 4], f32, tag="ut", bufs=4)
                nc.vector.tensor_scalar(u_t[:], var_t[:], -0.5, 1.5, ALU.mult, ALU.add)
                nwt = work.tile([128, 4], f32, tag="nwt", bufs=4)
                for _ in range(2):
                    nc.vector.tensor_tensor(nwt[:], u_t[:], u_t[:], op=ALU.mult)
                    nc.vector.tensor_tensor(nwt[:], nwt[:], var_t[:], op=ALU.mult)
                    nc.vector.tensor_scalar(nwt[:], nwt[:], -0.5, 1.5, ALU.mult, ALU.add)
                    nc.vector.tensor_tensor(u_t[:], u_t[:], nwt[:], op=ALU.mult)
                m_t = work.tile([128, 4], f32, tag="mt", bufs=4)
                nc.vector.tensor_tensor(m_t[:], mean_t[:], u_t[:], op=ALU.mult)
                ub_t = work.tile([128, 4], bf16, tag="ubt", bufs=4)
                mb_t = work.tile([128, 4], bf16, tag="mbt", bufs=4)
                nc.vector.tensor_copy(ub_t[:], u_t[:])
                nc.vector.tensor_copy(mb_t[:], m_t[:])
                nc.sync.dma_start(umT[0, :, isl * 4:(isl + 1) * 4], ub_t[:])
                nc.sync.dma_start(umT[1, :, isl * 4:(isl + 1) * 4], mb_t[:])
            # ---------------- QKV on raw x (LN affine folded into weights + correction) ----------------
            q2T = persist.tile([128, TOK], bf16, tag="q2T")
            k2T = persist.tile([128, TOK], bf16, tag="k2T")
            V2 = [None] * (B * JT)
            for isl in range(NSL):
                sl = slice(isl * 512, (isl + 1) * 512)
                b = isl // (NSL // B)
                ur = work.tile([1, 512], bf16, tag="umrow", bufs=4)
                nc.sync.dma_start(ur[:], um_d[0:1, sl])
                pu = pst()
                nc.tensor.matmul(pu[:], ones1[:], ur[:], start=True, stop=True)
                ub = persist.tile([128, 512], bf16, tag="Usb", bufs=NSL)
                nc.vector.tensor_copy(ub[:], pu[:])
                U_sb[isl] = ub
                mr = work.tile([1, 512], bf16, tag="umrow", bufs=4)
                nc.sync.dma_start(mr[:], um_d[1:2, sl])
                pm = pst()
                nc.tensor.matmul(pm[:], ones1[:], mr[:], start=True, stop=True)
                mb = work.tile([128, 512], bf16, tag="mbt2", bufs=2)
                nc.vector.tensor_copy(mb[:], pm[:])
                mu = persist.tile([128, 512], bf16, tag="MUsb", bufs=NSL)
                nc.vector.tensor_tensor(mu[:], ub[:], mb[:], op=ALU.mult)
                MU_sb[isl] = mu
                for p in (2, 1, 0):      # v first so V2 transposes start early
                    pq = pst()
                    for kt in range(KT):
                        nc.tensor.matmul(pq[:], wscaled[b][kt][:, p * 128:(p + 1) * 128],
                                         x_bf[kt][isl][:],
                                         start=(kt == 0), stop=(kt == KT - 1))
                    tq = work.tile([128, 512], bf16, tag="tq", bufs=4)
                    nc.vector.tensor_tensor(tq[:], pq[:], U_sb[isl][:], op=ALU.mult)
                    t2 = work.tile([128, 512], bf16, tag="tq2", bufs=4)
                    nc.vector.scalar_tensor_tensor(t2[:], MU_sb[isl][:], wgs_neg[b][:, p:p + 1],
                                                   tq[:], ALU.mult, ALU.add)
                    if p == 2:
                        vtile = work.tile([128, 512], bf16, tag="vtile", bufs=3)
                        nc.vector.tensor_scalar(vtile[:], t2[:], wbs[b][:, p:p + 1], None, ALU.add)
                        for q4 in range(4):
                            jt = isl * 4 + q4
                            pv = pst((128, 128), bf16)
                            nc.tensor.matmul(pv[:], vtile[:, q4 * 128:(q4 + 1) * 128],
                                             ident[:], is_transpose=True,
                                             start=True, stop=True)
                            va = persist.tile([128, 65], bf16, tag="Va0", bufs=B * JT)
                            nc.vector.tensor_copy(va[:, 0:64], pv[:, 0:64])
                            nc.vector.memset(va[:, 64:65], 1.0)
                            vh1 = persist.tile([128, 64], bf16, tag="Vh1", bufs=B * JT)
                            nc.vector.tensor_copy(vh1[:], pv[:, 64:128])
                            V2[jt] = (va, vh1)
                    elif p == 1:
                        nc.vector.tensor_scalar(k2T[:, sl], t2[:], wbs[b][:, p:p + 1], None, ALU.add)
                    else:
                        nc.vector.tensor_scalar(q2T[:, sl], t2[:], wbs[b][:, p:p + 1], None, ALU.add)

            # ---------------- attention (fused exp, forced pair adjacency) ----------------
            osb_all = {}
            for b in range(B):
                bo = b * N
                for isl in range(4):
                    po_h0 = pst()
                    po_h1 = pst()
                    pd1 = pst()
                    qsl = slice(bo + isl * 512, bo + (isl + 1) * 512)
                    for jt in range(JT):
                        ksl = slice(bo + jt * 128, bo + (jt + 1) * 128)
                        st2 = pst2()
                        nc.tensor.matmul(st2[:, 0:512], k2T[0:64, ksl], q2T[0:64, qsl],
                                         start=True, stop=True)
                        nc.tensor.matmul(st2[:, 512:1024], k2T[64:128, ksl], q2T[64:128, qsl],
                                         start=True, stop=True)
                        pt2 = work.tile([128, 1024], bf16, tag="pt2", bufs=7)
                        nc.scalar.activation(pt2[:], st2[:], AF.Exp, scale=DH ** -0.5)
                        gj = b * JT + jt
                        va, vh1 = V2[gj]
                        fl = (jt == 0), (jt == JT - 1)
                        nc.tensor.matmul(po_h0[0:65, :], va[:], pt2[:, 0:512],
                                         start=fl[0], stop=fl[1])
                        nc.tensor.matmul(po_h1[64:128, :], vh1[:], pt2[:, 512:1024],
                                         start=fl[0], stop=fl[1])
                        nc.tensor.matmul(pd1[32:33, :], ones_col[:], pt2[:, 512:1024],
                                         start=fl[0], stop=fl[1])
                    ob = persist.tile([128, 512], f32, tag="osb", bufs=8)
                    nc.vector.tensor_copy(ob[0:64, :], po_h0[0:64, :])
                    nc.vector.tensor_copy(ob[64:128, :], po_h1[64:128, :])
                    osb_all[(b, isl)] = ob
                    dstage = work.tile([128, 512], f32, tag="dstage", bufs=2)
                    nc.vector.tensor_copy(dstage[64:65, :], po_h0[64:65, :])
                    nc.vector.tensor_copy(dstage[32:33, :], pd1[32:33, :])
                    nc.sync.dma_start(den_d[b, isl, 0], dstage[64:65, :])
                    nc.sync.dma_start(den_d[b, isl, 1], dstage[32:33, :])

            # ---------------- normalize + output projection (after both attentions) ----------------
            o2t = persist.tile([128, TOK], bf16, tag="o2t")
            for b in range(B):
                bo = b * N
                denp = work.tile([8, 512], f32, tag="denp", bufs=1)
                nc.sync.dma_start(denp[:], den_d[b].rearrange("i h x -> (i h) x"))
                rp = work.tile([8, 512], f32, tag="rp", bufs=1)
                nc.vector.reciprocal(rp[:], denp[:])
                rpb = work.tile([8, 512], bf16, tag="rpb", bufs=2)
                nc.vector.tensor_copy(rpb[:], rp[:])
                nc.sync.dma_start(r_d[b].rearrange("i h x -> (i h) x"), rpb[:])
                for isl in range(4):
                    rp_isl = work.tile([2, 512], bf16, tag="rpisl", bufs=2)
                    nc.sync.dma_start(rp_isl[:], r_d[b].rearrange("i h x -> h i x")[:, isl:isl + 1])
                    pr = pst()
                    nc.tensor.matmul(pr[:], ones2[:], rp_isl[:], start=True, stop=True)
                    r2 = work.tile([128, 512], f32, tag="r2sb", bufs=1)
                    nc.vector.tensor_copy(r2[:], pr[:])
                    ob = osb_all[(b, isl)]
                    osl = slice(bo + isl * 512, bo + (isl + 1) * 512)
                    nc.vector.tensor_tensor(o2t[0:64, osl], ob[0:64, :], r2[0:64, :], op=ALU.mult)
                    nc.vector.tensor_tensor(o2t[64:128, osl], ob[64:128, :], r2[64:128, :], op=ALU.mult)
                for ncx in range(8):
                    for ts in range(4):
                        sl = slice(bo + ts * 512, bo + (ts + 1) * 512)
                        py = pst()
                        nc.tensor.matmul(py[:], wo_bf[:, ncx * 128:(ncx + 1) * 128],
                                         o2t[:, sl], start=True, stop=True)
                        yb = work.tile([128, 512], bf16, tag="ysb", bufs=3)
                        nc.scalar.copy(yb[:], py[:])
                        nc.sync.dma_start(yT_out[ncx * 128:(ncx + 1) * 128, sl], yb[:])

    nc.compile()
    return nc


_NC_CACHE = None


def _get_nc():
    global _NC_CACHE
    if _NC_CACHE is None:
        _NC_CACHE = build_program()
    return _NC_CACHE


def make_in_maps(x, conditioning_embeddings, gamma, cond_W, cond_b, Wq, Wkv, Wo):
    x = np.asarray(x, np.float32)
    ce = np.asarray(conditioning_embeddings, np.float32)
    gamma = np.asarray(gamma, np.float32)
    cond_W = np.asarray(cond_W, np.float32)
    cond_b = np.asarray(cond_b, np.float32)
    Wq = np.asarray(Wq, np.float32)
    Wkv = np.asarray(Wkv, np.float32)
    Wo = np.asarray(Wo, np.float32)

    bf = ml_dtypes.bfloat16
    xT = np.ascontiguousarray(x.reshape(TOK, DIM).T).astype(bf)
    ceT = np.ascontiguousarray(ce.reshape(B, KT, 128).transpose(2, 1, 0).reshape(128, 2 * KT))
    gammaT = np.ascontiguousarray(gamma.reshape(KT, 128).T)
    condb2 = np.ascontiguousarray(np.broadcast_to(cond_b, (2, 2 * DIM)))
    condW_bf = cond_W.astype(bf)
    ones2 = np.zeros((2, 128), np.float32)
    ones2[0, 0:64] = 1.0
    ones2[1, 64:128] = 1.0
    ones2 = ones2.astype(bf)

    in_maps = []
    for c in range(NCORES):
        cs = slice(128 * c, 128 * (c + 1))
        wqkv_c = np.ascontiguousarray(
            np.concatenate([Wq[:, cs], Wkv[:, cs], Wkv[:, 1024 + 128 * c:1024 + 128 * (c + 1)]], axis=1)
        ).astype(bf)
        in_maps.append({
            "xT": xT,
            "ceT": ceT,
            "gammaT": gammaT,
            "condW": condW_bf,
            "condb": condb2,
            "wqkv": wqkv_c,
            "wo": np.ascontiguousarray(Wo[cs, :]).astype(bf),
            "ones2": ones2,
        })
    return in_maps


def kernel(**inputs) -> np.ndarray:
    nc = _get_nc()
    in_maps = make_in_maps(**inputs)
    res = run_bass_kernel_spmd(nc, in_maps, core_ids=list(range(NCORES)))
    acc = np.zeros((DIM, TOK), np.float32)
    for core in res.results:
        acc += np.asarray(core["yT"]).astype(np.float32)
    return np.ascontiguousarray(acc.T).reshape(B, N, DIM)



# revision 12
# speedup vs baseline: 1.4624x; 1.4624x over previous
"""Trainium2 Bass kernel for nn_Attention_40037685133427.

FiLM-conditioned LayerNorm + 16-head self-attention (B=2, N=2048, D=1024),
tensor-parallel over 8 NeuronCores: core c owns heads {2c, 2c+1}.

v2 redesign, driven by microbenchmarks on this silicon:
  - back-to-back 512-col bf16 matmuls issue every ~216ns (and 64-contraction
    quadrant pairs co-execute), so the kernel is organized as long dependency-
    free PE streams; all DRAM round-trips (film/stat/denominator transposes)
    from v1 are replaced with on-chip PE transposes + partition-shifted DVE
    copies (both validated on HW).
  - LN stats: DVE pre-reduction over the 8 k-tiles (fp16 accumulators), one
    1-row PE matmul per slice for sum/sumsq, PE-transposed into token-major
    form, batched Newton rsqrt on DVE, PE ones-broadcast back.
  - h1's softmax denominator rides in the attn@V matmul via a 65-col
    stationary [V_h1 | ones] writing PSUM rows 0:65 (no separate ones-matmul);
    evacuation uses partition-shifted DVE copies.
  - normalize + output projection are emitted one slice late so their PE work
    never blocks the (ACT-exp-bound) attention stream.
Host sums the 8 partial y^T outputs (row-split Wo => partial sums).
"""

import sys

sys.path.insert(0, "/opt/trn_rl_repo")

import numpy as np
import ml_dtypes

import concourse.bass as bass
from concourse import bacc
import concourse.tile as tile
from concourse import mybir
from concourse.bass_utils import run_bass_kernel_spmd
from concourse.masks import make_identity

f32 = mybir.dt.float32
bf16 = mybir.dt.bfloat16
fp16 = mybir.dt.float16
AF = mybir.ActivationFunctionType
ALU = mybir.AluOpType

B, N, DIM = 2, 2048, 1024
HEADS, DH = 16, 64
TOK = B * N            # 4096 tokens, batch-major
KT = DIM // 128        # 8 k-tiles over the model dim
NSL = TOK // 512       # 8 token slices of 512
JT = N // 128          # 16 key tiles per batch
COND = 1024
NCORES = 8


def build_program():
    nc = bacc.Bacc("TRN2", target_bir_lowering=False, debug=False)

    xT = nc.dram_tensor("xT", [DIM, TOK], bf16, kind="ExternalInput").ap()
    ceT = nc.dram_tensor("ceT", [128, 2 * KT], f32, kind="ExternalInput").ap()
    gammaT = nc.dram_tensor("gammaT", [128, KT], f32, kind="ExternalInput").ap()
    condW = nc.dram_tensor("condW", [COND, 2 * DIM], bf16, kind="ExternalInput").ap()
    condb = nc.dram_tensor("condb", [2, 2 * DIM], f32, kind="ExternalInput").ap()
    wqkv = nc.dram_tensor("wqkv", [DIM, 384], bf16, kind="ExternalInput").ap()
    wo = nc.dram_tensor("wo", [128, DIM], bf16, kind="ExternalInput").ap()

    yT_out = nc.dram_tensor("yT", [DIM, TOK], bf16, kind="ExternalOutput").ap()

    with tile.TileContext(nc) as tc:
        with (
            tc.tile_pool(name="const", bufs=1) as const,
            tc.tile_pool(name="persist", bufs=1) as persist,
            tc.tile_pool(name="big", bufs=1) as bigp,
            tc.tile_pool(name="work", bufs=3) as work,
            tc.tile_pool(name="ps", bufs=8, space="PSUM") as ps,
        ):
            # PSUM tags: st2 2x[128,1024] (4 banks), po 2x[128,512] (2 banks),
            # ps 2x[128,512] (2 banks) = 8 banks
            def ps_st2(shape=(128, 1024), dtype=f32):
                return ps.tile(list(shape), dtype, tag="st2", bufs=2, name="st2t")

            def ps_po(shape=(128, 512), dtype=f32):
                return ps.tile(list(shape), dtype, tag="po", bufs=2, name="pot")

            def ps_sm(shape=(128, 512), dtype=f32):
                return ps.tile(list(shape), dtype, tag="ps", bufs=2, name="pst")

            # ---------------- constants / weights ----------------
            ident = const.tile([128, 128], bf16)
            make_identity(nc, ident[:])
            ident32 = const.tile([128, 128], f32)
            make_identity(nc, ident32[:])
            ones_col = const.tile([128, 1], bf16)
            nc.vector.memset(ones_col[:], 1.0)
            ones_col_h = const.tile([128, 1], fp16)
            nc.vector.memset(ones_col_h[:], 1.0)
            ones1 = const.tile([1, 128], bf16)
            nc.vector.memset(ones1[:], 1.0)
            # [33,128] selector: row 0 -> partitions 0:64 (head0), row 32 ->
            # 64:128 (head1); rows 1..31 are zero so junk moving rows vanish.
            ones2v = const.tile([33, 128], bf16)
            nc.vector.memset(ones2v[:], 0.0)
            nc.vector.memset(ones2v[0:1, 0:64], 1.0)
            nc.vector.memset(ones2v[32:33, 64:128], 1.0)

            wo_bf = persist.tile([128, DIM], bf16, tag="wo")
            nc.sync.dma_start(wo_bf[:], wo)
            wg_raw = []
            for kt in range(KT):
                wg = persist.tile([128, 384], bf16, tag="wg", bufs=KT, name=f"wg{kt}")
                nc.sync.dma_start(wg[:], wqkv[kt * 128:(kt + 1) * 128, :])
                wg_raw.append(wg)
            # x loaded as 16 [128, 2048] tiles: (kt, batch-half)
            x16 = [[None, None] for _ in range(KT)]
            for kt in range(KT):
                for bh in range(2):
                    xr = bigp.tile([128, 2048], bf16, tag="x16", bufs=16,
                                   name=f"x{kt}_{bh}")
                    eng = nc.sync if bh == 0 else nc.scalar
                    eng.dma_start(xr[:], xT[kt * 128:(kt + 1) * 128,
                                            bh * 2048:(bh + 1) * 2048])
                    x16[kt][bh] = xr

            def xsl(kt, isl):
                bh, q = isl // 4, isl % 4
                return x16[kt][bh][:, q * 512:(q + 1) * 512]

            gam = const.tile([128, KT], f32)
            nc.gpsimd.dma_start(gam[:], gammaT)
            cet = const.tile([128, 2 * KT], f32)
            nc.gpsimd.dma_start(cet[:], ceT)

            # ---------------- FiLM conditioning ----------------
            sil = const.tile([128, 2 * KT], f32)
            # silu(x) = x / (1 + exp(-x)) via Exp (single ACT table set)
            nc.scalar.activation(sil[:], cet[:], AF.Exp, scale=-1.0)
            nc.vector.tensor_scalar(sil[:], sil[:], 1.0, None, ALU.add)
            nc.vector.reciprocal(sil[:], sil[:])
            nc.vector.tensor_tensor(sil[:], sil[:], cet[:], op=ALU.mult)
            sil_bf = const.tile([128, 2 * KT], bf16)
            nc.vector.tensor_copy(sil_bf[:], sil[:])

            gp = const.tile([128, 2 * KT], f32)   # scale^T, col = b*KT + kt
            bp = const.tile([128, 2 * KT], f32)   # shift^T
            for cs in range(4):
                pc = ps_po((2, 512))
                for kt in range(KT):
                    cw = work.tile([128, 512], bf16, tag="cw", bufs=3)
                    nc.gpsimd.dma_start(cw[:], condW[kt * 128:(kt + 1) * 128,
                                                     cs * 512:(cs + 1) * 512])
                    nc.tensor.matmul(pc[:], sil_bf[:, 2 * kt:2 * kt + 2], cw[:],
                                     start=(kt == 0), stop=(kt == KT - 1))
                cbw = work.tile([2, 512], f32, tag="cbw", bufs=2)
                nc.gpsimd.dma_start(cbw[:], condb[:, cs * 512:(cs + 1) * 512])
                csl = work.tile([2, 512], f32, tag="csl", bufs=2)
                nc.vector.tensor_tensor(csl[:], pc[:], cbw[:], op=ALU.add)
                dst = gp if cs < 2 else bp
                for c in range(4):
                    tp = ps_sm((128, 2))
                    nc.tensor.matmul(tp[:], csl[0:2, c * 128:(c + 1) * 128],
                                     ident32[0:2, 0:2], is_transpose=True,
                                     start=True, stop=True)
                    k = (4 * cs + c) % KT
                    nc.vector.tensor_copy(dst[:, k::KT], tp[:])
            gpf = const.tile([128, 2 * KT], f32)
            nc.vector.tensor_scalar(gpf[:], gp[:], 1.0, None, ALU.add)
            for b in range(B):
                sl = slice(b * KT, (b + 1) * KT)
                nc.vector.tensor_tensor(gpf[:, sl], gpf[:, sl], gam[:], op=ALU.mult)
            bpb = const.tile([128, 2 * KT], bf16)
            nc.vector.tensor_copy(bpb[:], bp[:])

            # ---------------- per-batch folded QKV weights + column sums ----------------
            wscaled = [[None] * KT for _ in range(B)]
            wgs_neg, wbs = [], []
            for b in range(B):
                pgs = ps_po((1, 512))
                pbs = ps_po((1, 512))
                for kt in range(KT):
                    col = b * KT + kt
                    nc.tensor.matmul(pbs[0:1, 0:384], bpb[:, col:col + 1], wg_raw[kt][:],
                                     start=(kt == 0), stop=(kt == KT - 1))
                    wsb = persist.tile([128, 384], bf16, tag="wsb", bufs=B * KT,
                                       name=f"wsb{b}_{kt}")
                    nc.vector.tensor_scalar(wsb[:], wg_raw[kt][:], gpf[:, col:col + 1],
                                            None, ALU.mult)
                    nc.tensor.matmul(pgs[0:1, 0:384], ones_col[:], wsb[:],
                                     start=(kt == 0), stop=(kt == KT - 1))
                    wscaled[b][kt] = wsb
                gsr = work.tile([1, 512], f32, tag="wrow", bufs=2)
                nc.vector.tensor_copy(gsr[0:1, 0:384], pgs[0:1, 0:384])
                bsr = work.tile([1, 512], f32, tag="wrow", bufs=2)
                nc.vector.tensor_copy(bsr[0:1, 0:384], pbs[0:1, 0:384])
                wtp = ps_sm((128, 3))
                for c in range(3):
                    nc.tensor.matmul(wtp[:, c:c + 1], gsr[0:1, c * 128:(c + 1) * 128],
                                     ident32[0:1, 0:1], is_transpose=True,
                                     start=True, stop=True)
                wg_n = const.tile([128, 3], f32, name=f"wgn{b}")
                nc.vector.tensor_scalar(wg_n[:], wtp[:], -1.0, None, ALU.mult)
                wgs_neg.append(wg_n)
                wtp2 = ps_sm((128, 3))
                for c in range(3):
                    nc.tensor.matmul(wtp2[:, c:c + 1], bsr[0:1, c * 128:(c + 1) * 128],
                                     ident32[0:1, 0:1], is_transpose=True,
                                     start=True, stop=True)
                wb_c = const.tile([128, 3], f32, name=f"wbc{b}")
                nc.vector.tensor_copy(wb_c[:], wtp2[:])
                wbs.append(wb_c)

            # ---------------- LN stats: DVE pre-reduce + 1-row matmuls + transposes ----------------
            stag_su = [const.tile([128, 16], f32, name=f"stagsu{b}") for b in range(B)]
            stag_sq = [const.tile([128, 16], f32, name=f"stagsq{b}") for b in range(B)]
            for isl in range(NSL):
                b, q = isl // 4, isl % 4
                xs = work.tile([128, 512], fp16, tag="xs", bufs=2)
                xq = work.tile([128, 512], fp16, tag="xq", bufs=2)
                for kt in range(KT):
                    xb = xsl(kt, isl)
                    if kt == 0:
                        nc.vector.tensor_copy(xs[:], xb)
                        nc.vector.tensor_tensor(xq[:], xb, xb, op=ALU.mult)
                    else:
                        nc.vector.tensor_tensor(xs[:], xs[:], xb, op=ALU.add)
                        sq = work.tile([128, 512], fp16, tag="sq", bufs=2)
                        nc.vector.tensor_tensor(sq[:], xb, xb, op=ALU.mult)
                        nc.vector.tensor_tensor(xq[:], xq[:], sq[:], op=ALU.add)
                psu = ps_st2((1, 512))
                nc.tensor.matmul(psu[:], ones_col_h[:], xs[:], start=True, stop=True)
                psq = ps_st2((1, 512))
                nc.tensor.matmul(psq[:], ones_col_h[:], xq[:], start=True, stop=True)
                su_r = work.tile([1, 512], f32, tag="sr", bufs=2)
                nc.vector.tensor_copy(su_r[:], psu[:])
                sq_r = work.tile([1, 512], f32, tag="sr", bufs=2)
                nc.vector.tensor_copy(sq_r[:], psq[:])
                s2 = ps_sm((128, 8))
                for c in range(4):
                    nc.tensor.matmul(s2[:, c:c + 1],
                                     su_r[0:1, c * 128:(c + 1) * 128],
                                     ident32[0:1, 0:1], is_transpose=True,
                                     start=True, stop=True)
                    nc.tensor.matmul(s2[:, 4 + c:5 + c],
                                     sq_r[0:1, c * 128:(c + 1) * 128],
                                     ident32[0:1, 0:1], is_transpose=True,
                                     start=True, stop=True)
                nc.vector.tensor_copy(stag_su[b][:, q * 4:q * 4 + 4], s2[:, 0:4])
                nc.vector.tensor_copy(stag_sq[b][:, q * 4:q * 4 + 4], s2[:, 4:8])

            # ---------------- stage 2: batched Newton rsqrt + broadcast back ----------------
            U_sb, MU_sb = [None] * NSL, [None] * NSL
            for b in range(B):
                mean = work.tile([128, 16], f32, tag="mean", bufs=2)
                nc.vector.tensor_scalar(mean[:], stag_su[b][:], 1.0 / DIM, None, ALU.mult)
                var = work.tile([128, 16], f32, tag="var", bufs=2)
                nc.vector.tensor_scalar(var[:], stag_sq[b][:], 1.0 / DIM, 1e-5, ALU.mult, ALU.add)
                msq = work.tile([128, 16], f32, tag="msq", bufs=2)
                nc.vector.tensor_tensor(msq[:], mean[:], mean[:], op=ALU.mult)
                nc.vector.tensor_tensor(var[:], var[:], msq[:], op=ALU.subtract)
                y = work.tile([128, 16], f32, tag="yt", bufs=2)
                nc.vector.tensor_scalar(y[:], var[:], -0.5, 1.5, ALU.mult, ALU.add)
                t = work.tile([128, 16], f32, tag="tt", bufs=2)
                for _ in range(2):
                    nc.vector.tensor_tensor(t[:], y[:], y[:], op=ALU.mult)
                    nc.vector.tensor_tensor(t[:], t[:], var[:], op=ALU.mult)
                    nc.vector.tensor_scalar(t[:], t[:], -0.5, 1.5, ALU.mult, ALU.add)
                    nc.vector.tensor_tensor(y[:], y[:], t[:], op=ALU.mult)
                m = work.tile([128, 16], f32, tag="mt", bufs=2)
                nc.vector.tensor_tensor(m[:], mean[:], y[:], op=ALU.mult)
                stg2u = const.tile([128, 16], bf16, name=f"stg2u_{b}")
                nc.vector.tensor_copy(stg2u[:], y[:])
                stg2m = const.tile([128, 16], bf16, name=f"stg2m_{b}")
                nc.vector.tensor_copy(stg2m[:], m[:])
                for q in range(4):
                    isl = b * 4 + q
                    umps_u = ps_sm((1, 512), bf16)
                    umps_m = ps_sm((1, 512), bf16)
                    for c in range(4):
                        tt = q * 4 + c
                        nc.tensor.matmul(umps_u[0:1, c * 128:(c + 1) * 128],
                                         stg2u[:, tt:tt + 1], ident[:],
                                         is_transpose=True, start=True, stop=True)
                        nc.tensor.matmul(umps_m[0:1, c * 128:(c + 1) * 128],
                                         stg2m[:, tt:tt + 1], ident[:],
                                         is_transpose=True, start=True, stop=True)
                    ur = work.tile([1, 512], bf16, tag="umrow", bufs=2)
                    nc.vector.tensor_copy(ur[:], umps_u[:])
                    mr = work.tile([1, 512], bf16, tag="umrow", bufs=2)
                    nc.vector.tensor_copy(mr[:], umps_m[:])
                    pu = ps_sm()
                    nc.tensor.matmul(pu[:], ones1[:], ur[:], start=True, stop=True)
                    ub = persist.tile([128, 512], bf16, tag="Usb", bufs=NSL)
                    nc.vector.tensor_copy(ub[:], pu[:])
                    U_sb[isl] = ub
                    pm = ps_sm()
                    nc.tensor.matmul(pm[:], ones1[:], mr[:], start=True, stop=True)
                    mu = persist.tile([128, 512], bf16, tag="MUsb", bufs=NSL)
                    nc.vector.tensor_tensor(mu[:], ub[:], pm[:], op=ALU.mult)
                    MU_sb[isl] = mu

            # ---------------- QKV on raw x (LN+FiLM affine folded into weights) ----------------
            q2T = persist.tile([128, TOK], bf16, tag="q2T")
            k2T = persist.tile([128, TOK], bf16, tag="k2T")
            V2 = [None] * (B * JT)
            for isl in range(NSL):
                sl = slice(isl * 512, (isl + 1) * 512)
                b = isl // (NSL // B)
                for p in (2, 1, 0):      # v first so V2 transposes start early
                    pq = ps_po()
                    for kt in range(KT):
                        nc.tensor.matmul(pq[:], wscaled[b][kt][:, p * 128:(p + 1) * 128],
                                         xsl(kt, isl),
                                         start=(kt == 0), stop=(kt == KT - 1))
                    tq = work.tile([128, 512], bf16, tag="tq", bufs=4)
                    nc.vector.tensor_tensor(tq[:], pq[:], U_sb[isl][:], op=ALU.mult)
                    t2 = work.tile([128, 512], bf16, tag="tq2", bufs=4)
                    nc.vector.scalar_tensor_tensor(t2[:], MU_sb[isl][:], wgs_neg[b][:, p:p + 1],
                                                   tq[:], ALU.mult, ALU.add)
                    if p == 2:
                        vtile = work.tile([128, 512], bf16, tag="vtile", bufs=3)
                        nc.vector.tensor_scalar(vtile[:], t2[:], wbs[b][:, p:p + 1], None, ALU.add)
                        for q4 in range(4):
                            jt = isl * 4 + q4
                            pv = ps_sm((128, 128), bf16)
                            nc.tensor.matmul(pv[:], vtile[:, q4 * 128:(q4 + 1) * 128],
                                             ident[:], is_transpose=True,
                                             start=True, stop=True)
                            va = persist.tile([128, 65], bf16, tag="Va0", bufs=B * JT)
                            nc.vector.tensor_copy(va[:, 0:64], pv[:, 0:64])
                            nc.vector.memset(va[:, 64:65], 1.0)
                            vh = persist.tile([128, 65], bf16, tag="Vh1", bufs=B * JT)
                            nc.vector.tensor_copy(vh[:, 0:64], pv[:, 64:128])
                            nc.vector.memset(vh[:, 64:65], 1.0)
                            V2[jt] = (va, vh)
                    elif p == 1:
                        nc.vector.tensor_scalar(k2T[:, sl], t2[:], wbs[b][:, p:p + 1], None, ALU.add)
                    else:
                        nc.vector.tensor_scalar(q2T[:, sl], t2[:], wbs[b][:, p:p + 1], None, ALU.add)

            # ---------------- attention + late normalize/outproj ----------------
            o2t = persist.tile([128, TOK], bf16, tag="o2t")

            def attn_isl(b, isl):
                bo = b * N
                po_h0 = ps_po()
                po_h1 = ps_po()
                qsl = slice(bo + isl * 512, bo + (isl + 1) * 512)
                for jt in range(JT):
                    ksl = slice(bo + jt * 128, bo + (jt + 1) * 128)
                    st2 = ps_st2()
                    nc.tensor.matmul(st2[:, 0:512], k2T[0:64, ksl], q2T[0:64, qsl],
                                     start=True, stop=True)
                    nc.tensor.matmul(st2[:, 512:1024], k2T[64:128, ksl], q2T[64:128, qsl],
                                     start=True, stop=True)
                    pt2 = work.tile([128, 1024], bf16, tag="pt2", bufs=4)
                    nc.scalar.activation(pt2[:], st2[:], AF.Exp, scale=DH ** -0.5)
                    va, vh = V2[b * JT + jt]
                    fl = (jt == 0), (jt == JT - 1)
                    nc.tensor.matmul(po_h0[0:65, :], va[:], pt2[:, 0:512],
                                     start=fl[0], stop=fl[1])
                    nc.tensor.matmul(po_h1[0:65, :], vh[:], pt2[:, 512:1024],
                                     start=fl[0], stop=fl[1])
                return po_h0, po_h1

            def evac(po_h0, po_h1):
                ob = work.tile([128, 512], f32, tag="ob", bufs=2)
                nc.vector.tensor_copy(ob[0:64, :], po_h0[0:64, :])
                nc.vector.tensor_copy(ob[64:128, :], po_h1[0:64, :])   # shifted
                rin0 = work.tile([1, 512], f32, tag="rin", bufs=2)
                nc.vector.tensor_copy(rin0[:], po_h0[64:65, :])        # shifted
                rin1 = work.tile([1, 512], f32, tag="rin", bufs=2)
                nc.vector.tensor_copy(rin1[:], po_h1[64:65, :])        # shifted
                rp0 = work.tile([1, 512], f32, tag="rp", bufs=2)
                nc.vector.reciprocal(rp0[:], rin0[:])
                rp1 = work.tile([1, 512], f32, tag="rp", bufs=2)
                nc.vector.reciprocal(rp1[:], rin1[:])
                # moving rows for the r-broadcast matmul live at partitions 0
                # and 32 (32-aligned); rows 1..31 zeroed, killed by ones2v=0.
                rpb = work.tile([33, 512], bf16, tag="rpb", bufs=2)
                nc.vector.memset(rpb[0:32, :], 0.0)
                nc.vector.tensor_copy(rpb[0:1, :], rp0[:])
                nc.vector.tensor_copy(rpb[32:33, :], rp1[:])
                return ob, rpb

            def normout(b, isl, ob, rpb):
                bo = b * N
                osl = slice(bo + isl * 512, bo + (isl + 1) * 512)
                pr = ps_sm()
                nc.tensor.matmul(pr[:], ones2v[:], rpb[:], start=True, stop=True)
                nc.vector.tensor_tensor(o2t[0:64, osl], ob[0:64, :], pr[0:64, :], op=ALU.mult)
                nc.vector.tensor_tensor(o2t[64:128, osl], ob[64:128, :], pr[64:128, :], op=ALU.mult)
                for ncx in range(8):
                    py = ps_sm()
                    nc.tensor.matmul(py[:], wo_bf[:, ncx * 128:(ncx + 1) * 128],
                                     o2t[:, osl], start=True, stop=True)
                    yb = work.tile([128, 512], bf16, tag="ysb", bufs=3)
                    nc.vector.tensor_copy(yb[:], py[:])
                    nc.sync.dma_start(yT_out[ncx * 128:(ncx + 1) * 128, osl], yb[:])

            pending = None
            for b in range(B):
                for isl in range(4):
                    po_h0, po_h1 = attn_isl(b, isl)
                    e = evac(po_h0, po_h1)
                    if pending is not None:
                        normout(*pending)
                    pending = (b, isl, e[0], e[1])
            normout(*pending)

    nc.compile()
    return nc


_NC_CACHE = None


def _get_nc():
    global _NC_CACHE
    if _NC_CACHE is None:
        _NC_CACHE = build_program()
    return _NC_CACHE


def make_in_maps(x, conditioning_embeddings, gamma, cond_W, cond_b, Wq, Wkv, Wo):
    x = np.asarray(x, np.float32)
    ce = np.asarray(conditioning_embeddings, np.float32)
    gamma = np.asarray(gamma, np.float32)
    cond_W = np.asarray(cond_W, np.float32)
    cond_b = np.asarray(cond_b, np.float32)
    Wq = np.asarray(Wq, np.float32)
    Wkv = np.asarray(Wkv, np.float32)
    Wo = np.asarray(Wo, np.float32)

    bf = ml_dtypes.bfloat16
    xT = np.ascontiguousarray(x.reshape(TOK, DIM).T).astype(bf)
    ceT = np.ascontiguousarray(ce.reshape(B, KT, 128).transpose(2, 1, 0).reshape(128, 2 * KT))
    gammaT = np.ascontiguousarray(gamma.reshape(KT, 128).T)
    condb2 = np.ascontiguousarray(np.broadcast_to(cond_b, (2, 2 * DIM)))
    condW_bf = cond_W.astype(bf)
    in_maps = []
    for c in range(NCORES):
        cs = slice(128 * c, 128 * (c + 1))
        wqkv_c = np.ascontiguousarray(
            np.concatenate([Wq[:, cs], Wkv[:, cs], Wkv[:, 1024 + 128 * c:1024 + 128 * (c + 1)]], axis=1)
        ).astype(bf)
        in_maps.append({
            "xT": xT,
            "ceT": ceT,
            "gammaT": gammaT,
            "condW": condW_bf,
            "condb": condb2,
            "wqkv": wqkv_c,
            "wo": np.ascontiguousarray(Wo[cs, :]).astype(bf),
        })
    return in_maps


def kernel(**inputs) -> np.ndarray:
    nc = _get_nc()
    in_maps = make_in_maps(**inputs)
    res = run_bass_kernel_spmd(nc, in_maps, core_ids=list(range(NCORES)))
    acc = np.zeros((DIM, TOK), np.float32)
    for core in res.results:
        acc += np.asarray(core["yT"]).astype(np.float32)
    return np.ascontiguousarray(acc.T).reshape(B, N, DIM)


# revision 21
# speedup vs baseline: 1.7244x; 1.1792x over previous
"""Trainium2 Bass kernel for nn_Attention_40037685133427.

FiLM-conditioned LayerNorm + 16-head self-attention (B=2, N=2048, D=1024),
tensor-parallel over 8 NeuronCores: core c owns heads {2c, 2c+1}.

v2 redesign, driven by microbenchmarks on this silicon:
  - back-to-back 512-col bf16 matmuls issue every ~216ns (and 64-contraction
    quadrant pairs co-execute), so the kernel is organized as long dependency-
    free PE streams; all DRAM round-trips (film/stat/denominator transposes)
    from v1 are replaced with on-chip PE transposes + partition-shifted DVE
    copies (both validated on HW).
  - LN stats: DVE pre-reduction over the 8 k-tiles (fp16 accumulators), one
    1-row PE matmul per slice for sum/sumsq, PE-transposed into token-major
    form, batched Newton rsqrt on DVE, PE ones-broadcast back.
  - h1's softmax denominator rides in the attn@V matmul via a 65-col
    stationary [V_h1 | ones] writing PSUM rows 0:65 (no separate ones-matmul);
    evacuation uses partition-shifted DVE copies.
  - normalize + output projection are emitted one slice late so their PE work
    never blocks the (ACT-exp-bound) attention stream.
Host sums the 8 partial y^T outputs (row-split Wo => partial sums).
"""

import sys

sys.path.insert(0, "/opt/trn_rl_repo")

import numpy as np
import ml_dtypes

import concourse.bass as bass
from concourse import bacc
import concourse.tile as tile
from concourse import mybir
from concourse.bass_utils import run_bass_kernel_spmd
from concourse.masks import make_identity

f32 = mybir.dt.float32
bf16 = mybir.dt.bfloat16
fp16 = mybir.dt.float16
AF = mybir.ActivationFunctionType
ALU = mybir.AluOpType

B, N, DIM = 2, 2048, 1024
HEADS, DH = 16, 64
TOK = B * N            # 4096 tokens, batch-major
KT = DIM // 128        # 8 k-tiles over the model dim
NSL = TOK // 512       # 8 token slices of 512
JT = N // 128          # 16 key tiles per batch
COND = 1024
NCORES = 8


def build_program():
    nc = bacc.Bacc("TRN2", target_bir_lowering=False, debug=False)

    xT = nc.dram_tensor("xT", [DIM, TOK], bf16, kind="ExternalInput").ap()
    xN = nc.dram_tensor("xN", [TOK, DIM], bf16, kind="ExternalInput").ap()
    ceT = nc.dram_tensor("ceT", [128, 2 * KT], f32, kind="ExternalInput").ap()
    gammaT = nc.dram_tensor("gammaT", [128, KT], f32, kind="ExternalInput").ap()
    condW = nc.dram_tensor("condW", [COND, 2 * DIM], bf16, kind="ExternalInput").ap()
    condb = nc.dram_tensor("condb", [2, 2 * DIM], f32, kind="ExternalInput").ap()
    wqkv = nc.dram_tensor("wqkv", [DIM, 384], bf16, kind="ExternalInput").ap()
    wo = nc.dram_tensor("wo", [128, DIM], bf16, kind="ExternalInput").ap()

    yT_out = nc.dram_tensor("yT", [DIM, TOK], bf16, kind="ExternalOutput").ap()

    with tile.TileContext(nc) as tc:
        with (
            tc.tile_pool(name="const", bufs=1) as const,
            tc.tile_pool(name="persist", bufs=1) as persist,
            tc.tile_pool(name="big", bufs=1) as bigp,
            tc.tile_pool(name="work", bufs=3) as work,
            tc.tile_pool(name="ps", bufs=8, space="PSUM") as ps,
        ):
            # PSUM tags: st2 2x[128,1024] (4 banks), po 2x[128,512] (2 banks),
            # ps 2x[128,512] (2 banks) = 8 banks
            def ps_st2(shape=(128, 1024), dtype=f32):
                return ps.tile(list(shape), dtype, tag="st2", bufs=2, name="st2t")

            def ps_po(shape=(128, 512), dtype=f32):
                return ps.tile(list(shape), dtype, tag="po", bufs=2, name="pot")

            def ps_sm(shape=(128, 512), dtype=f32):
                return ps.tile(list(shape), dtype, tag="ps", bufs=2, name="pst")

            # ---------------- constants / weights ----------------
            ident = const.tile([128, 128], bf16)
            make_identity(nc, ident[:])
            ident32 = const.tile([128, 128], f32)
            make_identity(nc, ident32[:])
            ones_col = const.tile([128, 1], bf16)
            nc.vector.memset(ones_col[:], 1.0)
            ones_col_h = const.tile([128, 1], fp16)
            nc.vector.memset(ones_col_h[:], 1.0)
            ones1 = const.tile([1, 128], bf16)
            nc.vector.memset(ones1[:], 1.0)
            # [33,128] selector: row 0 -> partitions 0:64 (head0), row 32 ->
            # 64:128 (head1); rows 1..31 are zero so junk moving rows vanish.
            ones2v = const.tile([33, 128], bf16)
            nc.vector.memset(ones2v[:], 0.0)
            nc.vector.memset(ones2v[0:1, 0:64], 1.0)
            nc.vector.memset(ones2v[32:33, 64:128], 1.0)

            wo_bf = persist.tile([128, DIM], bf16, tag="wo")
            nc.sync.dma_start(wo_bf[:], wo)
            wg_raw = []
            for kt in range(KT):
                wg = persist.tile([128, 384], bf16, tag="wg", bufs=KT, name=f"wg{kt}")
                nc.sync.dma_start(wg[:], wqkv[kt * 128:(kt + 1) * 128, :])
                wg_raw.append(wg)
            # x loaded as 16 [128, 2048] tiles: (kt, batch-half)
            x16 = [[None, None] for _ in range(KT)]
            for kt in range(KT):
                for bh in range(2):
                    xr = bigp.tile([128, 2048], bf16, tag="x16", bufs=16,
                                   name=f"x{kt}_{bh}")
                    eng = nc.sync if bh == 0 else nc.scalar
                    eng.dma_start(xr[:], xT[kt * 128:(kt + 1) * 128,
                                            bh * 2048:(bh + 1) * 2048])
                    x16[kt][bh] = xr

            def xsl(kt, isl):
                bh, q = isl // 4, isl % 4
                return x16[kt][bh][:, q * 512:(q + 1) * 512]

            gam = const.tile([128, KT], f32)
            nc.gpsimd.dma_start(gam[:], gammaT)
            cet = const.tile([128, 2 * KT], f32)
            nc.gpsimd.dma_start(cet[:], ceT)

            # ---------------- FiLM conditioning ----------------
            sil = const.tile([128, 2 * KT], f32)
            # silu(x) = x / (1 + exp(-x)) via Exp (single ACT table set)
            nc.scalar.activation(sil[:], cet[:], AF.Exp, scale=-1.0)
            nc.vector.tensor_scalar(sil[:], sil[:], 1.0, None, ALU.add)
            nc.vector.reciprocal(sil[:], sil[:])
            nc.vector.tensor_tensor(sil[:], sil[:], cet[:], op=ALU.mult)
            sil_bf = const.tile([128, 2 * KT], bf16)
            nc.vector.tensor_copy(sil_bf[:], sil[:])

            gp = const.tile([128, 2 * KT], f32)   # scale^T, col = b*KT + kt
            bp = const.tile([128, 2 * KT], f32)   # shift^T
            for cs in range(4):
                pc = ps_po((2, 512))
                for kt in range(KT):
                    cw = work.tile([128, 512], bf16, tag="cw", bufs=3)
                    nc.gpsimd.dma_start(cw[:], condW[kt * 128:(kt + 1) * 128,
                                                     cs * 512:(cs + 1) * 512])
                    nc.tensor.matmul(pc[:], sil_bf[:, 2 * kt:2 * kt + 2], cw[:],
                                     start=(kt == 0), stop=(kt == KT - 1))
                cbw = work.tile([2, 512], f32, tag="cbw", bufs=2)
                nc.gpsimd.dma_start(cbw[:], condb[:, cs * 512:(cs + 1) * 512])
                csl = work.tile([2, 512], f32, tag="csl", bufs=2)
                nc.vector.tensor_tensor(csl[:], pc[:], cbw[:], op=ALU.add)
                dst = gp if cs < 2 else bp
                for c in range(4):
                    tp = ps_sm((128, 2))
                    nc.tensor.matmul(tp[:], csl[0:2, c * 128:(c + 1) * 128],
                                     ident32[0:2, 0:2], is_transpose=True,
                                     start=True, stop=True)
                    k = (4 * cs + c) % KT
                    nc.vector.tensor_copy(dst[:, k::KT], tp[:])
            gpf = const.tile([128, 2 * KT], f32)
            nc.vector.tensor_scalar(gpf[:], gp[:], 1.0, None, ALU.add)
            for b in range(B):
                sl = slice(b * KT, (b + 1) * KT)
                nc.vector.tensor_tensor(gpf[:, sl], gpf[:, sl], gam[:], op=ALU.mult)
            bpb = const.tile([128, 2 * KT], bf16)
            nc.vector.tensor_copy(bpb[:], bp[:])

            # ---------------- per-batch folded QKV weights + column sums ----------------
            wscaled = [[None] * KT for _ in range(B)]
            wgs_neg, wbs = [], []
            for b in range(B):
                pgs = ps_po((1, 512))
                pbs = ps_po((1, 512))
                for kt in range(KT):
                    col = b * KT + kt
                    nc.tensor.matmul(pbs[0:1, 0:384], bpb[:, col:col + 1], wg_raw[kt][:],
                                     start=(kt == 0), stop=(kt == KT - 1))
                    wsb = persist.tile([128, 384], bf16, tag="wsb", bufs=B * KT,
                                       name=f"wsb{b}_{kt}")
                    nc.vector.tensor_scalar(wsb[:], wg_raw[kt][:], gpf[:, col:col + 1],
                                            None, ALU.mult)
                    nc.tensor.matmul(pgs[0:1, 0:384], ones_col[:], wsb[:],
                                     start=(kt == 0), stop=(kt == KT - 1))
                    wscaled[b][kt] = wsb
                gsr = work.tile([1, 512], f32, tag="wrow", bufs=2)
                nc.vector.tensor_copy(gsr[0:1, 0:384], pgs[0:1, 0:384])
                bsr = work.tile([1, 512], f32, tag="wrow", bufs=2)
                nc.vector.tensor_copy(bsr[0:1, 0:384], pbs[0:1, 0:384])
                wtp = ps_sm((128, 3))
                for c in range(3):
                    nc.tensor.matmul(wtp[:, c:c + 1], gsr[0:1, c * 128:(c + 1) * 128],
                                     ident32[0:1, 0:1], is_transpose=True,
                                     start=True, stop=True)
                wg_n = const.tile([128, 3], f32, name=f"wgn{b}")
                nc.vector.tensor_scalar(wg_n[:], wtp[:], -1.0, None, ALU.mult)
                wgs_neg.append(wg_n)
                wtp2 = ps_sm((128, 3))
                for c in range(3):
                    nc.tensor.matmul(wtp2[:, c:c + 1], bsr[0:1, c * 128:(c + 1) * 128],
                                     ident32[0:1, 0:1], is_transpose=True,
                                     start=True, stop=True)
                wb_c = const.tile([128, 3], f32, name=f"wbc{b}")
                nc.vector.tensor_copy(wb_c[:], wtp2[:])
                wbs.append(wb_c)

            # ---------------- LN stats via DVE bn_stats on token-major x ----------------
            # bn_stats/bn_aggr produce per-token (mean, var) directly; no PE
            # matmuls, no serial accumulation chains, no transposes needed.
            stagMV = [const.tile([128, 32], f32, name=f"stagmv{b}") for b in range(B)]

            def stats_b(b):
                for q in range(16):          # 16 token-tiles of 128 per batch
                    tt = b * 16 + q
                    xn = work.tile([128, DIM], bf16, tag="xn", bufs=3)
                    nc.sync.dma_start(xn[:], xN[tt * 128:(tt + 1) * 128, :])
                    bnb = work.tile([128, 12], f32, tag="bnb", bufs=3)
                    nc.vector.bn_stats(bnb[:, 0:6], xn[:, 0:512])
                    nc.vector.bn_stats(bnb[:, 6:12], xn[:, 512:1024])
                    nc.vector.bn_aggr(stagMV[b][:, 2 * q:2 * q + 2], bnb[:])

            # ---------------- stage 2: batched Newton rsqrt + broadcast back ----------------
            U_sb, MU_sb = [None] * NSL, [None] * NSL

            def stage2_b(b):
                mean = work.tile([128, 16], f32, tag="mean", bufs=2)
                nc.vector.tensor_copy(mean[:], stagMV[b][:, 0::2])
                var = work.tile([128, 16], f32, tag="var", bufs=2)
                nc.vector.tensor_scalar(var[:], stagMV[b][:, 1::2], 1.0, 1e-5, ALU.mult, ALU.add)
                y = work.tile([128, 16], f32, tag="yt", bufs=2)
                nc.vector.tensor_scalar(y[:], var[:], -0.5, 1.5, ALU.mult, ALU.add)
                t = work.tile([128, 16], f32, tag="tt", bufs=2)
                for _ in range(2):
                    nc.vector.tensor_tensor(t[:], y[:], y[:], op=ALU.mult)
                    nc.vector.tensor_tensor(t[:], t[:], var[:], op=ALU.mult)
                    nc.vector.tensor_scalar(t[:], t[:], -0.5, 1.5, ALU.mult, ALU.add)
                    nc.vector.tensor_tensor(y[:], y[:], t[:], op=ALU.mult)
                m = work.tile([128, 16], f32, tag="mt", bufs=2)
                nc.vector.tensor_tensor(m[:], mean[:], y[:], op=ALU.mult)
                stg2u = const.tile([128, 16], bf16, name=f"stg2u_{b}")
                nc.vector.tensor_copy(stg2u[:], y[:])
                stg2m = const.tile([128, 16], bf16, name=f"stg2m_{b}")
                nc.vector.tensor_copy(stg2m[:], m[:])
                for q in range(4):
                    isl = b * 4 + q
                    umps_u = ps_sm((1, 512), bf16)
                    umps_m = ps_sm((1, 512), bf16)
                    for c in range(4):
                        tt = q * 4 + c
                        nc.tensor.matmul(umps_u[0:1, c * 128:(c + 1) * 128],
                                         stg2u[:, tt:tt + 1], ident[:],
                                         is_transpose=True, start=True, stop=True)
                        nc.tensor.matmul(umps_m[0:1, c * 128:(c + 1) * 128],
                                         stg2m[:, tt:tt + 1], ident[:],
                                         is_transpose=True, start=True, stop=True)
                    ur = work.tile([1, 512], bf16, tag="umrow", bufs=2)
                    nc.vector.tensor_copy(ur[:], umps_u[:])
                    mr = work.tile([1, 512], bf16, tag="umrow", bufs=2)
                    nc.vector.tensor_copy(mr[:], umps_m[:])
                    pu = ps_sm()
                    nc.tensor.matmul(pu[:], ones1[:], ur[:], start=True, stop=True)
                    ub = persist.tile([128, 512], bf16, tag="Usb", bufs=NSL)
                    nc.vector.tensor_copy(ub[:], pu[:])
                    U_sb[isl] = ub
                    pm = ps_sm()
                    nc.tensor.matmul(pm[:], ones1[:], mr[:], start=True, stop=True)
                    mu = persist.tile([128, 512], bf16, tag="MUsb", bufs=NSL)
                    nc.vector.tensor_tensor(mu[:], ub[:], pm[:], op=ALU.mult)
                    MU_sb[isl] = mu

            # ---------------- QKV on raw x (LN+FiLM affine folded into weights) ----------------
            q2T = persist.tile([128, TOK], bf16, tag="q2T")
            k2T = persist.tile([128, TOK], bf16, tag="k2T")
            V2 = [None] * (B * JT)

            def qkv_isl(isl):
                sl = slice(isl * 512, (isl + 1) * 512)
                b = isl // (NSL // B)
                for p in (2, 1, 0):      # v first so V2 transposes start early
                    pq = ps_po()
                    for kt in range(KT):
                        nc.tensor.matmul(pq[:], wscaled[b][kt][:, p * 128:(p + 1) * 128],
                                         xsl(kt, isl),
                                         start=(kt == 0), stop=(kt == KT - 1))
                    tq = work.tile([128, 512], bf16, tag="tq", bufs=4)
                    nc.vector.tensor_tensor(tq[:], pq[:], U_sb[isl][:], op=ALU.mult)
                    t2 = work.tile([128, 512], bf16, tag="tq2", bufs=4)
                    nc.vector.scalar_tensor_tensor(t2[:], MU_sb[isl][:], wgs_neg[b][:, p:p + 1],
                                                   tq[:], ALU.mult, ALU.add)
                    if p == 2:
                        vtile = work.tile([128, 512], bf16, tag="vtile", bufs=3)
                        nc.vector.tensor_scalar(vtile[:], t2[:], wbs[b][:, p:p + 1], None, ALU.add)
                        for q4 in range(4):
                            jt = isl * 4 + q4
                            pv = ps_sm((128, 128), bf16)
                            nc.tensor.matmul(pv[:], vtile[:, q4 * 128:(q4 + 1) * 128],
                                             ident[:], is_transpose=True,
                                             start=True, stop=True)
                            va = persist.tile([128, 65], bf16, tag="Va0", bufs=B * JT)
                            nc.vector.tensor_copy(va[:, 0:64], pv[:, 0:64])
                            nc.vector.memset(va[:, 64:65], 1.0)
                            vh = persist.tile([128, 65], bf16, tag="Vh1", bufs=B * JT)
                            nc.vector.tensor_copy(vh[:, 0:64], pv[:, 64:128])
                            nc.vector.memset(vh[:, 64:65], 1.0)
                            V2[jt] = (va, vh)
                    elif p == 1:
                        nc.vector.tensor_scalar(k2T[:, sl], t2[:], wbs[b][:, p:p + 1], None, ALU.add)
                    else:
                        nc.vector.tensor_scalar(q2T[:, sl], t2[:], wbs[b][:, p:p + 1], None, ALU.add)

            # stats(b) runs on DVE while qkv of the previous batch streams on PE
            for b in range(B):
                stats_b(b)
                stage2_b(b)
                for q in range(4):
                    qkv_isl(b * 4 + q)

            # ---------------- attention + late normalize/outproj ----------------
            o2t = persist.tile([128, TOK], bf16, tag="o2t")

            def attn_isl(b, isl):
                bo = b * N
                po_h0 = ps_po()
                po_h1 = ps_po()
                qsl = slice(bo + isl * 512, bo + (isl + 1) * 512)
                for jt in range(JT):
                    ksl = slice(bo + jt * 128, bo + (jt + 1) * 128)
                    st2 = ps_st2()
                    nc.tensor.matmul(st2[:, 0:512], k2T[0:64, ksl], q2T[0:64, qsl],
                                     start=True, stop=True)
                    nc.tensor.matmul(st2[:, 512:1024], k2T[64:128, ksl], q2T[64:128, qsl],
                                     start=True, stop=True)
                    pt2 = work.tile([128, 1024], bf16, tag="pt2", bufs=4)
                    nc.scalar.activation(pt2[:], st2[:], AF.Exp, scale=DH ** -0.5)
                    va, vh = V2[b * JT + jt]
                    fl = (jt == 0), (jt == JT - 1)
                    nc.tensor.matmul(po_h0[0:65, :], va[:], pt2[:, 0:512],
                                     start=fl[0], stop=fl[1])
                    nc.tensor.matmul(po_h1[0:65, :], vh[:], pt2[:, 512:1024],
                                     start=fl[0], stop=fl[1])
                return po_h0, po_h1

            def evac(po_h0, po_h1):
                ob = work.tile([128, 512], bf16, tag="ob", bufs=2)
                nc.vector.tensor_copy(ob[0:64, :], po_h0[0:64, :])
                nc.vector.tensor_copy(ob[64:128, :], po_h1[0:64, :])   # shifted
                rin0 = work.tile([1, 512], f32, tag="rin", bufs=2)
                nc.vector.tensor_copy(rin0[:], po_h0[64:65, :])        # shifted
                rin1 = work.tile([1, 512], f32, tag="rin", bufs=2)
                nc.vector.tensor_copy(rin1[:], po_h1[64:65, :])        # shifted
                rp0 = work.tile([1, 512], f32, tag="rp", bufs=2)
                nc.vector.reciprocal_approx_fast(rp0[:], rin0[:])
                rp1 = work.tile([1, 512], f32, tag="rp", bufs=2)
                nc.vector.reciprocal_approx_fast(rp1[:], rin1[:])
                # moving rows for the r-broadcast matmul live at partitions 0
                # and 32 (32-aligned); rows 1..31 zeroed, killed by ones2v=0.
                rpb = work.tile([33, 512], bf16, tag="rpb", bufs=2)
                nc.vector.memset(rpb[0:32, :], 0.0)
                nc.vector.tensor_copy(rpb[0:1, :], rp0[:])
                nc.vector.tensor_copy(rpb[32:33, :], rp1[:])
                return ob, rpb

            def normout(b, isl, ob, rpb):
                bo = b * N
                osl = slice(bo + isl * 512, bo + (isl + 1) * 512)
                pr = ps_sm()
                nc.tensor.matmul(pr[:], ones2v[:], rpb[:], start=True, stop=True)
                nc.vector.tensor_tensor(o2t[0:64, osl], ob[0:64, :], pr[0:64, :], op=ALU.mult)
                nc.vector.tensor_tensor(o2t[64:128, osl], ob[64:128, :], pr[64:128, :], op=ALU.mult)
                for ncx in range(8):
                    py = ps_sm()
                    nc.tensor.matmul(py[:], wo_bf[:, ncx * 128:(ncx + 1) * 128],
                                     o2t[:, osl], start=True, stop=True)
                    yb = work.tile([128, 512], bf16, tag="ysb", bufs=3)
                    nc.vector.tensor_copy(yb[:], py[:])
                    nc.sync.dma_start(yT_out[ncx * 128:(ncx + 1) * 128, osl], yb[:])

            pending = None
            for b in range(B):
                for isl in range(4):
                    po_h0, po_h1 = attn_isl(b, isl)
                    if pending is not None:
                        normout(*pending)
                    e = evac(po_h0, po_h1)
                    pending = (b, isl, e[0], e[1])
            normout(*pending)

    nc.compile()
    return nc


_NC_CACHE = None


def _get_nc():
    global _NC_CACHE
    if _NC_CACHE is None:
        _NC_CACHE = build_program()
    return _NC_CACHE


def make_in_maps(x, conditioning_embeddings, gamma, cond_W, cond_b, Wq, Wkv, Wo):
    x = np.asarray(x, np.float32)
    ce = np.asarray(conditioning_embeddings, np.float32)
    gamma = np.asarray(gamma, np.float32)
    cond_W = np.asarray(cond_W, np.float32)
    cond_b = np.asarray(cond_b, np.float32)
    Wq = np.asarray(Wq, np.float32)
    Wkv = np.asarray(Wkv, np.float32)
    Wo = np.asarray(Wo, np.float32)

    bf = ml_dtypes.bfloat16
    xT = np.ascontiguousarray(x.reshape(TOK, DIM).T).astype(bf)
    xN = np.ascontiguousarray(x.reshape(TOK, DIM)).astype(bf)
    ceT = np.ascontiguousarray(ce.reshape(B, KT, 128).transpose(2, 1, 0).reshape(128, 2 * KT))
    gammaT = np.ascontiguousarray(gamma.reshape(KT, 128).T)
    condb2 = np.ascontiguousarray(np.broadcast_to(cond_b, (2, 2 * DIM)))
    condW_bf = cond_W.astype(bf)
    in_maps = []
    for c in range(NCORES):
        cs = slice(128 * c, 128 * (c + 1))
        wqkv_c = np.ascontiguousarray(
            np.concatenate([Wq[:, cs], Wkv[:, cs], Wkv[:, 1024 + 128 * c:1024 + 128 * (c + 1)]], axis=1)
        ).astype(bf)
        in_maps.append({
            "xT": xT,
            "xN": xN,
            "ceT": ceT,
            "gammaT": gammaT,
            "condW": condW_bf,
            "condb": condb2,
            "wqkv": wqkv_c,
            "wo": np.ascontiguousarray(Wo[cs, :]).astype(bf),
        })
    return in_maps


def kernel(**inputs) -> np.ndarray:
    nc = _get_nc()
    in_maps = make_in_maps(**inputs)
    res = run_bass_kernel_spmd(nc, in_maps, core_ids=list(range(NCORES)))
    acc = np.zeros((DIM, TOK), np.float32)
    for core in res.results:
        acc += np.asarray(core["yT"]).astype(np.float32)
    return np.ascontiguousarray(acc.T).reshape(B, N, DIM)


# revision 26
# speedup vs baseline: 1.8058x; 1.0472x over previous
"""Trainium2 Bass kernel for nn_Attention_40037685133427.

FiLM-conditioned LayerNorm + 16-head self-attention (B=2, N=2048, D=1024),
tensor-parallel over 8 NeuronCores: core c owns heads {2c, 2c+1}.

v2 redesign, driven by microbenchmarks on this silicon:
  - back-to-back 512-col bf16 matmuls issue every ~216ns (and 64-contraction
    quadrant pairs co-execute), so the kernel is organized as long dependency-
    free PE streams; all DRAM round-trips (film/stat/denominator transposes)
    from v1 are replaced with on-chip PE transposes + partition-shifted DVE
    copies (both validated on HW).
  - LN stats: DVE pre-reduction over the 8 k-tiles (fp16 accumulators), one
    1-row PE matmul per slice for sum/sumsq, PE-transposed into token-major
    form, batched Newton rsqrt on DVE, PE ones-broadcast back.
  - h1's softmax denominator rides in the attn@V matmul via a 65-col
    stationary [V_h1 | ones] writing PSUM rows 0:65 (no separate ones-matmul);
    evacuation uses partition-shifted DVE copies.
  - normalize + output projection are emitted one slice late so their PE work
    never blocks the (ACT-exp-bound) attention stream.
Host sums the 8 partial y^T outputs (row-split Wo => partial sums).
"""

import sys

sys.path.insert(0, "/opt/trn_rl_repo")

import numpy as np
import ml_dtypes

import concourse.bass as bass
from concourse import bacc
import concourse.tile as tile
from concourse import mybir
from concourse.bass_utils import run_bass_kernel_spmd
from concourse.masks import make_identity

f32 = mybir.dt.float32
bf16 = mybir.dt.bfloat16
fp16 = mybir.dt.float16
AF = mybir.ActivationFunctionType
ALU = mybir.AluOpType

B, N, DIM = 2, 2048, 1024
HEADS, DH = 16, 64
TOK = B * N            # 4096 tokens, batch-major
KT = DIM // 128        # 8 k-tiles over the model dim
NSL = TOK // 512       # 8 token slices of 512
JT = N // 128          # 16 key tiles per batch
COND = 1024
NCORES = 8


def build_program():
    nc = bacc.Bacc("TRN2", target_bir_lowering=False, debug=False)

    xT = nc.dram_tensor("xT", [DIM, TOK], bf16, kind="ExternalInput").ap()
    xN = nc.dram_tensor("xN", [TOK, DIM], bf16, kind="ExternalInput").ap()
    ceT = nc.dram_tensor("ceT", [128, 2 * KT], f32, kind="ExternalInput").ap()
    gammaT = nc.dram_tensor("gammaT", [128, KT], f32, kind="ExternalInput").ap()
    condW = nc.dram_tensor("condW", [COND, 2 * DIM], bf16, kind="ExternalInput").ap()
    condb = nc.dram_tensor("condb", [2, 2 * DIM], f32, kind="ExternalInput").ap()
    wqkv = nc.dram_tensor("wqkv", [DIM, 384], bf16, kind="ExternalInput").ap()
    wo = nc.dram_tensor("wo", [128, DIM], bf16, kind="ExternalInput").ap()

    yT_out = nc.dram_tensor("yT", [DIM, TOK], bf16, kind="ExternalOutput").ap()

    with tile.TileContext(nc) as tc:
        with (
            tc.tile_pool(name="const", bufs=1) as const,
            tc.tile_pool(name="persist", bufs=1) as persist,
            tc.tile_pool(name="big", bufs=1) as bigp,
            tc.tile_pool(name="work", bufs=3) as work,
            tc.tile_pool(name="ps", bufs=8, space="PSUM") as ps,
        ):
            # PSUM tags: st2 2x[128,1024] (4 banks), po 2x[128,512] (2 banks),
            # ps 2x[128,512] (2 banks) = 8 banks
            def ps_st2(shape=(128, 1024), dtype=f32):
                return ps.tile(list(shape), dtype, tag="st2", bufs=2, name="st2t")

            def ps_po(shape=(128, 512), dtype=f32):
                return ps.tile(list(shape), dtype, tag="po", bufs=2, name="pot")

            def ps_sm(shape=(128, 512), dtype=f32):
                return ps.tile(list(shape), dtype, tag="ps", bufs=2, name="pst")

            # ---------------- constants / weights ----------------
            ident = const.tile([128, 128], bf16)
            make_identity(nc, ident[:])
            ident32 = const.tile([128, 128], f32)
            make_identity(nc, ident32[:])
            ones_col = const.tile([128, 1], bf16)
            nc.vector.memset(ones_col[:], 1.0)
            ones_col_h = const.tile([128, 1], fp16)
            nc.vector.memset(ones_col_h[:], 1.0)
            ones1 = const.tile([1, 128], bf16)
            nc.vector.memset(ones1[:], 1.0)
            # [33,128] selector: row 0 -> partitions 0:64 (head0), row 32 ->
            # 64:128 (head1); rows 1..31 are zero so junk moving rows vanish.
            ones2v = const.tile([33, 128], bf16)
            nc.vector.memset(ones2v[:], 0.0)
            nc.vector.memset(ones2v[0:1, 0:64], 1.0)
            nc.vector.memset(ones2v[32:33, 64:128], 1.0)

            wo_bf = persist.tile([128, DIM], bf16, tag="wo")
            nc.sync.dma_start(wo_bf[:], wo)
            wg_raw = []
            for kt in range(KT):
                wg = persist.tile([128, 384], bf16, tag="wg", bufs=KT, name=f"wg{kt}")
                nc.sync.dma_start(wg[:], wqkv[kt * 128:(kt + 1) * 128, :])
                wg_raw.append(wg)
            # x loaded as 16 [128, 2048] tiles: (kt, batch-half)
            x16 = [[None, None] for _ in range(KT)]
            for kt in range(KT):
                for bh in range(2):
                    xr = bigp.tile([128, 2048], bf16, tag="x16", bufs=16,
                                   name=f"x{kt}_{bh}")
                    eng = nc.sync if bh == 0 else nc.scalar
                    eng.dma_start(xr[:], xT[kt * 128:(kt + 1) * 128,
                                            bh * 2048:(bh + 1) * 2048])
                    x16[kt][bh] = xr

            def xsl(kt, isl):
                bh, q = isl // 4, isl % 4
                return x16[kt][bh][:, q * 512:(q + 1) * 512]

            gam = const.tile([128, KT], f32)
            nc.gpsimd.dma_start(gam[:], gammaT)
            cet = const.tile([128, 2 * KT], f32)
            nc.gpsimd.dma_start(cet[:], ceT)

            # ---------------- FiLM conditioning ----------------
            sil = const.tile([128, 2 * KT], f32)
            # silu(x) = x / (1 + exp(-x)) via Exp (single ACT table set)
            nc.scalar.activation(sil[:], cet[:], AF.Exp, scale=-1.0)
            nc.vector.tensor_scalar(sil[:], sil[:], 1.0, None, ALU.add)
            nc.vector.reciprocal(sil[:], sil[:])
            nc.vector.tensor_tensor(sil[:], sil[:], cet[:], op=ALU.mult)
            sil_bf = const.tile([128, 2 * KT], bf16)
            nc.vector.tensor_copy(sil_bf[:], sil[:])

            gp = const.tile([128, 2 * KT], f32)   # scale^T, col = b*KT + kt
            bp = const.tile([128, 2 * KT], f32)   # shift^T
            for cs in range(4):
                pc = ps_po((2, 512))
                for kt in range(KT):
                    cw = work.tile([128, 512], bf16, tag="cw", bufs=3)
                    nc.gpsimd.dma_start(cw[:], condW[kt * 128:(kt + 1) * 128,
                                                     cs * 512:(cs + 1) * 512])
                    nc.tensor.matmul(pc[:], sil_bf[:, 2 * kt:2 * kt + 2], cw[:],
                                     start=(kt == 0), stop=(kt == KT - 1))
                cbw = work.tile([2, 512], f32, tag="cbw", bufs=2)
                nc.gpsimd.dma_start(cbw[:], condb[:, cs * 512:(cs + 1) * 512])
                csl = work.tile([2, 512], f32, tag="csl", bufs=2)
                nc.vector.tensor_tensor(csl[:], pc[:], cbw[:], op=ALU.add)
                dst = gp if cs < 2 else bp
                for c in range(4):
                    tp = ps_sm((128, 2))
                    nc.tensor.matmul(tp[:], csl[0:2, c * 128:(c + 1) * 128],
                                     ident32[0:2, 0:2], is_transpose=True,
                                     start=True, stop=True)
                    k = (4 * cs + c) % KT
                    nc.vector.tensor_copy(dst[:, k::KT], tp[:])
            gpf = const.tile([128, 2 * KT], f32)
            nc.vector.tensor_scalar(gpf[:], gp[:], 1.0, None, ALU.add)
            for b in range(B):
                sl = slice(b * KT, (b + 1) * KT)
                nc.vector.tensor_tensor(gpf[:, sl], gpf[:, sl], gam[:], op=ALU.mult)
            bpb = const.tile([128, 2 * KT], bf16)
            nc.vector.tensor_copy(bpb[:], bp[:])

            # ---------------- per-batch folded QKV weights + column sums ----------------
            wscaled = [[None] * KT for _ in range(B)]
            wgs_neg, wbs = [], []
            for b in range(B):
                pgs = ps_po((1, 512))
                pbs = ps_po((1, 512))
                for kt in range(KT):
                    col = b * KT + kt
                    nc.tensor.matmul(pbs[0:1, 0:384], bpb[:, col:col + 1], wg_raw[kt][:],
                                     start=(kt == 0), stop=(kt == KT - 1))
                    wsb = persist.tile([128, 384], bf16, tag="wsb", bufs=B * KT,
                                       name=f"wsb{b}_{kt}")
                    nc.vector.tensor_scalar(wsb[:], wg_raw[kt][:], gpf[:, col:col + 1],
                                            None, ALU.mult)
                    nc.tensor.matmul(pgs[0:1, 0:384], ones_col[:], wsb[:],
                                     start=(kt == 0), stop=(kt == KT - 1))
                    wscaled[b][kt] = wsb
                gsr = work.tile([1, 512], f32, tag="wrow", bufs=2)
                nc.vector.tensor_copy(gsr[0:1, 0:384], pgs[0:1, 0:384])
                bsr = work.tile([1, 512], f32, tag="wrow", bufs=2)
                nc.vector.tensor_copy(bsr[0:1, 0:384], pbs[0:1, 0:384])
                wtp = ps_sm((128, 3))
                for c in range(3):
                    nc.tensor.matmul(wtp[:, c:c + 1], gsr[0:1, c * 128:(c + 1) * 128],
                                     ident32[0:1, 0:1], is_transpose=True,
                                     start=True, stop=True)
                wg_n = const.tile([128, 3], f32, name=f"wgn{b}")
                nc.vector.tensor_scalar(wg_n[:], wtp[:], -1.0, None, ALU.mult)
                wgs_neg.append(wg_n)
                wtp2 = ps_sm((128, 3))
                for c in range(3):
                    nc.tensor.matmul(wtp2[:, c:c + 1], bsr[0:1, c * 128:(c + 1) * 128],
                                     ident32[0:1, 0:1], is_transpose=True,
                                     start=True, stop=True)
                wb_c = const.tile([128, 3], f32, name=f"wbc{b}")
                nc.vector.tensor_copy(wb_c[:], wtp2[:])
                wbs.append(wb_c)

            # ---------------- LN stats via DVE bn_stats on token-major x ----------------
            # bn_stats/bn_aggr produce per-token (mean, var) directly; no PE
            # matmuls, no serial accumulation chains, no transposes needed.
            stagMV = [const.tile([128, 32], f32, name=f"stagmv{b}") for b in range(B)]

            def stats_b(b):
                for q in range(16):          # 16 token-tiles of 128 per batch
                    tt = b * 16 + q
                    xn = work.tile([128, DIM], bf16, tag="xn", bufs=6)
                    nc.scalar.dma_start(xn[:], xN[tt * 128:(tt + 1) * 128, :])
                    bnb = work.tile([128, 12], f32, tag="bnb", bufs=3)
                    nc.vector.bn_stats(bnb[:, 0:6], xn[:, 0:512])
                    nc.vector.bn_stats(bnb[:, 6:12], xn[:, 512:1024])
                    nc.vector.bn_aggr(stagMV[b][:, 2 * q:2 * q + 2], bnb[:])

            # ---------------- stage 2: batched Newton rsqrt + broadcast back ----------------
            U_sb, MU_sb = [None] * NSL, [None] * NSL
            # [2,128] row-selector matrices, built by PE transpose of columns
            # (rows past 0 can't be written directly by engines)
            selc = const.tile([128, 4], bf16)
            nc.vector.memset(selc[:, 0:1], 1.0)
            nc.vector.memset(selc[:, 1:2], 0.0)
            nc.vector.memset(selc[:, 2:3], 0.0)
            nc.vector.memset(selc[:, 3:4], 1.0)
            sel_u = const.tile([2, 128], bf16)
            sel_m = const.tile([2, 128], bf16)
            for sel, c0 in ((sel_u, 0), (sel_m, 2)):
                psel = ps_sm((2, 128), bf16)
                nc.tensor.matmul(psel[:], selc[:, c0:c0 + 2], ident[:],
                                 is_transpose=True, start=True, stop=True)
                nc.vector.tensor_copy(sel[:], psel[:])

            def stage2_b(b):
                mean = work.tile([128, 16], f32, tag="mean", bufs=2)
                nc.vector.tensor_copy(mean[:], stagMV[b][:, 0::2])
                var = work.tile([128, 16], f32, tag="var", bufs=2)
                nc.vector.tensor_scalar(var[:], stagMV[b][:, 1::2], 1.0, 1e-5, ALU.mult, ALU.add)
                y = work.tile([128, 16], f32, tag="yt", bufs=2)
                nc.vector.tensor_scalar(y[:], var[:], -0.5, 1.5, ALU.mult, ALU.add)
                t = work.tile([128, 16], f32, tag="tt", bufs=2)
                for _ in range(2):
                    nc.vector.tensor_tensor(t[:], y[:], y[:], op=ALU.mult)
                    nc.vector.tensor_tensor(t[:], t[:], var[:], op=ALU.mult)
                    nc.vector.tensor_scalar(t[:], t[:], -0.5, 1.5, ALU.mult, ALU.add)
                    nc.vector.tensor_tensor(y[:], y[:], t[:], op=ALU.mult)
                m = work.tile([128, 16], f32, tag="mt", bufs=2)
                nc.vector.tensor_tensor(m[:], mean[:], y[:], op=ALU.mult)
                stg2 = const.tile([128, 32], bf16, name=f"stg2_{b}")
                nc.vector.tensor_copy(stg2[:, 0::2], y[:])
                nc.vector.tensor_copy(stg2[:, 1::2], m[:])
                for q in range(4):
                    isl = b * 4 + q
                    umps = ps_sm((2, 512), bf16)
                    for c in range(4):
                        tt = q * 4 + c
                        nc.tensor.matmul(umps[0:2, c * 128:(c + 1) * 128],
                                         stg2[:, 2 * tt:2 * tt + 2], ident[:],
                                         is_transpose=True, start=True, stop=True)
                    umr = work.tile([2, 512], bf16, tag="umrow", bufs=2)
                    nc.vector.tensor_copy(umr[:], umps[:])
                    pu = ps_sm()
                    nc.tensor.matmul(pu[:], sel_u[:], umr[:], start=True, stop=True)
                    ub = persist.tile([128, 512], bf16, tag="Usb", bufs=NSL)
                    nc.vector.tensor_copy(ub[:], pu[:])
                    U_sb[isl] = ub
                    pm = ps_sm()
                    nc.tensor.matmul(pm[:], sel_m[:], umr[:], start=True, stop=True)
                    mu = persist.tile([128, 512], bf16, tag="MUsb", bufs=NSL)
                    nc.vector.tensor_tensor(mu[:], ub[:], pm[:], op=ALU.mult)
                    MU_sb[isl] = mu

            # ---------------- QKV on raw x (LN+FiLM affine folded into weights) ----------------
            q2T = persist.tile([128, TOK], bf16, tag="q2T")
            k2T = persist.tile([128, TOK], bf16, tag="k2T")
            V2 = [None] * (B * JT)

            def qkv_isl(isl):
                sl = slice(isl * 512, (isl + 1) * 512)
                b = isl // (NSL // B)
                # 3 interleaved accumulation chains (separate PSUM banks) so
                # consecutive matmuls are independent and stream back-to-back
                pqs = {2: ps_po(), 1: ps_po(), 0: ps_sm()}
                for kt in range(KT):
                    for p in (2, 1, 0):
                        nc.tensor.matmul(pqs[p][:], wscaled[b][kt][:, p * 128:(p + 1) * 128],
                                         xsl(kt, isl),
                                         start=(kt == 0), stop=(kt == KT - 1))
                for p in (2, 1, 0):      # v first so V2 transposes start early
                    pq = pqs[p]
                    tq = work.tile([128, 512], bf16, tag="tq", bufs=4)
                    nc.vector.tensor_tensor(tq[:], pq[:], U_sb[isl][:], op=ALU.mult)
                    t2 = work.tile([128, 512], bf16, tag="tq2", bufs=4)
                    nc.vector.scalar_tensor_tensor(t2[:], MU_sb[isl][:], wgs_neg[b][:, p:p + 1],
                                                   tq[:], ALU.mult, ALU.add)
                    if p == 2:
                        vtile = work.tile([128, 512], bf16, tag="vtile", bufs=3)
                        nc.vector.tensor_scalar(vtile[:], t2[:], wbs[b][:, p:p + 1], None, ALU.add)
                        for q4 in range(4):
                            jt = isl * 4 + q4
                            pv = ps_sm((128, 128), bf16)
                            nc.tensor.matmul(pv[:], vtile[:, q4 * 128:(q4 + 1) * 128],
                                             ident[:], is_transpose=True,
                                             start=True, stop=True)
                            va = persist.tile([128, 65], bf16, tag="Va0", bufs=B * JT)
                            nc.vector.tensor_copy(va[:, 0:64], pv[:, 0:64])
                            nc.vector.memset(va[:, 64:65], 1.0)
                            vh = persist.tile([128, 65], bf16, tag="Vh1", bufs=B * JT)
                            nc.vector.tensor_copy(vh[:, 0:64], pv[:, 64:128])
                            nc.vector.memset(vh[:, 64:65], 1.0)
                            V2[jt] = (va, vh)
                    elif p == 1:
                        nc.vector.tensor_scalar(k2T[:, sl], t2[:], wbs[b][:, p:p + 1], None, ALU.add)
                    else:
                        nc.vector.tensor_scalar(q2T[:, sl], t2[:], wbs[b][:, p:p + 1], None, ALU.add)

            # stats run on DVE/DMA queues; PE streams film/wscaled/QKV under them
            stats_b(0)
            stats_b(1)
            stage2_b(0)
            for q in range(4):
                qkv_isl(q)
            stage2_b(1)
            for q in range(4):
                qkv_isl(4 + q)

            # ---------------- attention + late normalize/outproj ----------------
            o2t = persist.tile([128, TOK], bf16, tag="o2t")

            def attn_isl(b, isl):
                bo = b * N
                po_h0 = ps_po()
                po_h1 = ps_po()
                qsl = slice(bo + isl * 512, bo + (isl + 1) * 512)
                for jt in range(JT):
                    ksl = slice(bo + jt * 128, bo + (jt + 1) * 128)
                    st2 = ps_st2()
                    nc.tensor.matmul(st2[:, 0:512], k2T[0:64, ksl], q2T[0:64, qsl],
                                     start=True, stop=True)
                    nc.tensor.matmul(st2[:, 512:1024], k2T[64:128, ksl], q2T[64:128, qsl],
                                     start=True, stop=True)
                    pt2 = work.tile([128, 1024], bf16, tag="pt2", bufs=4)
                    nc.scalar.activation(pt2[:], st2[:], AF.Exp, scale=DH ** -0.5)
                    va, vh = V2[b * JT + jt]
                    fl = (jt == 0), (jt == JT - 1)
                    nc.tensor.matmul(po_h0[0:65, :], va[:], pt2[:, 0:512],
                                     start=fl[0], stop=fl[1])
                    nc.tensor.matmul(po_h1[0:65, :], vh[:], pt2[:, 512:1024],
                                     start=fl[0], stop=fl[1])
                return po_h0, po_h1

            def evac(po_h0, po_h1):
                ob = work.tile([128, 512], bf16, tag="ob", bufs=2)
                nc.vector.tensor_copy(ob[0:64, :], po_h0[0:64, :])
                nc.vector.tensor_copy(ob[64:128, :], po_h1[0:64, :])   # shifted
                rin0 = work.tile([1, 512], f32, tag="rin", bufs=2)
                nc.vector.tensor_copy(rin0[:], po_h0[64:65, :])        # shifted
                rin1 = work.tile([1, 512], f32, tag="rin", bufs=2)
                nc.vector.tensor_copy(rin1[:], po_h1[64:65, :])        # shifted
                rp0 = work.tile([1, 512], f32, tag="rp", bufs=2)
                nc.vector.reciprocal_approx_fast(rp0[:], rin0[:])
                rp1 = work.tile([1, 512], f32, tag="rp", bufs=2)
                nc.vector.reciprocal_approx_fast(rp1[:], rin1[:])
                # moving rows for the r-broadcast matmul live at partitions 0
                # and 32 (32-aligned); rows 1..31 zeroed, killed by ones2v=0.
                rpb = work.tile([33, 512], bf16, tag="rpb", bufs=2)
                nc.vector.memset(rpb[0:32, :], 0.0)
                nc.vector.tensor_copy(rpb[0:1, :], rp0[:])
                nc.vector.tensor_copy(rpb[32:33, :], rp1[:])
                return ob, rpb

            def normout(b, isl, ob, rpb):
                bo = b * N
                osl = slice(bo + isl * 512, bo + (isl + 1) * 512)
                pr = ps_sm()
                nc.tensor.matmul(pr[:], ones2v[:], rpb[:], start=True, stop=True)
                nc.vector.tensor_tensor(o2t[0:64, osl], ob[0:64, :], pr[0:64, :], op=ALU.mult)
                nc.vector.tensor_tensor(o2t[64:128, osl], ob[64:128, :], pr[64:128, :], op=ALU.mult)
                for ncx in range(8):
                    py = ps_sm()
                    nc.tensor.matmul(py[:], wo_bf[:, ncx * 128:(ncx + 1) * 128],
                                     o2t[:, osl], start=True, stop=True)
                    yb = work.tile([128, 512], bf16, tag="ysb", bufs=3)
                    nc.vector.tensor_copy(yb[:], py[:])
                    nc.sync.dma_start(yT_out[ncx * 128:(ncx + 1) * 128, osl], yb[:])

            pending = None
            for b in range(B):
                for isl in range(4):
                    po_h0, po_h1 = attn_isl(b, isl)
                    if pending is not None:
                        normout(*pending)
                    e = evac(po_h0, po_h1)
                    pending = (b, isl, e[0], e[1])
            normout(*pending)

    nc.compile()
    return nc


_NC_CACHE = None


def _get_nc():
    global _NC_CACHE
    if _NC_CACHE is None:
        _NC_CACHE = build_program()
    return _NC_CACHE


def make_in_maps(x, conditioning_embeddings, gamma, cond_W, cond_b, Wq, Wkv, Wo):
    x = np.asarray(x, np.float32)
    ce = np.asarray(conditioning_embeddings, np.float32)
    gamma = np.asarray(gamma, np.float32)
    cond_W = np.asarray(cond_W, np.float32)
    cond_b = np.asarray(cond_b, np.float32)
    Wq = np.asarray(Wq, np.float32)
    Wkv = np.asarray(Wkv, np.float32)
    Wo = np.asarray(Wo, np.float32)

    bf = ml_dtypes.bfloat16
    xT = np.ascontiguousarray(x.reshape(TOK, DIM).T).astype(bf)
    xN = np.ascontiguousarray(x.reshape(TOK, DIM)).astype(bf)
    ceT = np.ascontiguousarray(ce.reshape(B, KT, 128).transpose(2, 1, 0).reshape(128, 2 * KT))
    gammaT = np.ascontiguousarray(gamma.reshape(KT, 128).T)
    condb2 = np.ascontiguousarray(np.broadcast_to(cond_b, (2, 2 * DIM)))
    condW_bf = cond_W.astype(bf)
    in_maps = []
    for c in range(NCORES):
        cs = slice(128 * c, 128 * (c + 1))
        wqkv_c = np.ascontiguousarray(
            np.concatenate([Wq[:, cs], Wkv[:, cs], Wkv[:, 1024 + 128 * c:1024 + 128 * (c + 1)]], axis=1)
        ).astype(bf)
        in_maps.append({
            "xT": xT,
            "xN": xN,
            "ceT": ceT,
            "gammaT": gammaT,
            "condW": condW_bf,
            "condb": condb2,
            "wqkv": wqkv_c,
            "wo": np.ascontiguousarray(Wo[cs, :]).astype(bf),
        })
    return in_maps


def kernel(**inputs) -> np.ndarray:
    nc = _get_nc()
    in_maps = make_in_maps(**inputs)
    res = run_bass_kernel_spmd(nc, in_maps, core_ids=list(range(NCORES)))
    acc = np.zeros((DIM, TOK), np.float32)
    for core in res.results:
        acc += np.asarray(core["yT"]).astype(np.float32)
    return np.ascontiguousarray(acc.T).reshape(B, N, DIM)
